# revision 7
# baseline (speedup 1.0000x reference)
"""CorefScore kernel for 8 Trainium2 NeuronCores.

Shards the mention axis M=2048 across 8 cores (256 mentions each plus a
64-row halo of preceding mentions). Per core, the banded pairwise MLP is
computed as 50 shifted elementwise products X^T * shift(X^T, delta) (DVE,
fp16, batched 8 deltas per op) contracted with W1c on the PE in fp16 with
fp32 PSUM accumulation; the Ya + shift(Yb) term is merged on DVE and added
into PSUM via an identity matmul. ReLU activations are stored in large SBUF
buffers; the w2p contraction runs as a deferred back-to-back matmul phase so
the PE stream never waits on ScalarE mid-round. Masking/dummy column are
applied with host-precomputed mask tensors.
"""

import os
import sys

import numpy as np

for _p in ("/opt/trn_rl_repo", "/opt/pypackages"):
    if os.path.isdir(_p) and _p not in sys.path:
        sys.path.append(_p)

import concourse.bacc as bacc
import concourse.bass as bass
import concourse.mybir as mybir
import concourse.tile as tile
from concourse.ap import AP
from concourse.bass_utils import run_bass_kernel_spmd

F16 = mybir.dt.float16
F32 = mybir.dt.float32
AF = mybir.ActivationFunctionType

M, D, H, K = 2048, 900, 150, 50
NCORES = 8
MC = M // NCORES          # owned mentions per core
HB = 64                   # halo columns (>= K)
W = MC + HB               # X^T window width per core
DP = 1024                 # padded feature dim (8 tiles of 128)
NDT = DP // 128           # number of d tiles
G = 2                     # deltas per PSUM round
NR = K // G               # rounds
GP = 8                    # deltas per product batch
H1, H2 = 128, H - 128     # h split
KM = K * MC

_cache = {}


def _ap3(t_ap, p_lo, p_n, off, dims):
    """3-D free-dim view of a tile AP: partitions [p_lo, p_lo+p_n), free
    offset `off` elements, free dims = [(stride, n), ...]."""
    b = t_ap[p_lo:p_lo + p_n, 0:1]
    pstride = b.ap[0][0]
    return AP(b.tensor, b.offset + off, [[pstride, p_n]] + [list(d) for d in dims])


def _build():
    nc = bacc.Bacc("TRN2", target_bir_lowering=False, debug=False)

    xt_d = nc.dram_tensor("xt", [DP, W], F16, kind="ExternalInput").ap()
    w1c_d = nc.dram_tensor("w1c", [DP, H], F16, kind="ExternalInput").ap()
    w1a_d = nc.dram_tensor("w1a", [DP, H], F16, kind="ExternalInput").ap()
    w1b_d = nc.dram_tensor("w1b", [DP, H], F16, kind="ExternalInput").ap()
    w1m_d = nc.dram_tensor("w1m", [DP, H], F16, kind="ExternalInput").ap()
    w2m_d = nc.dram_tensor("w2m", [H, 1], F16, kind="ExternalInput").ap()
    w2p1_d = nc.dram_tensor("w2p1", [H1, 1], F16, kind="ExternalInput").ap()
    w2p2_d = nc.dram_tensor("w2p2", [33, 1], F16, kind="ExternalInput").ap()
    idn_d = nc.dram_tensor("idn", [128, 128], F16, kind="ExternalInput").ap()
    b1m_d = nc.dram_tensor("b1mc", [H, 1], F32, kind="ExternalInput").ap()
    b1p_d = nc.dram_tensor("b1pc", [H, 1], F32, kind="ExternalInput").ap()
    mmul_d = nc.dram_tensor("mmul", [MC, K + 1], F32, kind="ExternalInput").ap()
    madd_d = nc.dram_tensor("madd", [MC, K + 1], F32, kind="ExternalInput").ap()
    out_d = nc.dram_tensor("out", [MC, K + 1], F32, kind="ExternalOutput").ap()

    hsl = [(0, H1), (H1, H2)]  # (h offset, h size) per h tile

    with tile.TileContext(nc) as tc:
        with (
            tc.tile_pool(name="const", bufs=1) as cp,
            tc.tile_pool(name="work", bufs=2) as wp,
            tc.tile_pool(name="ps_pre", bufs=2, space="PSUM") as pp_pre,
            tc.tile_pool(name="ps_a1", bufs=2, space="PSUM") as pp_a1,
            tc.tile_pool(name="ps_a2", bufs=2, space="PSUM") as pp_a2,
            tc.tile_pool(name="ps_pair", bufs=2, space="PSUM") as pp_pair,
        ):
            # ---- load inputs ----
            xts = []
            for t in range(NDT):
                xt = cp.tile([128, W], F16, tag=f"xt{t}")
                nc.sync.dma_start(out=xt[:], in_=xt_d[128 * t:128 * (t + 1), :])
                xts.append(xt)

            def load_w(dram, name):
                ts = []
                for t in range(NDT):
                    w = cp.tile([128, H], F16, tag=f"{name}{t}")
                    nc.sync.dma_start(out=w[:], in_=dram[128 * t:128 * (t + 1), :])
                    ts.append(w)
                return ts

            w1c_s = load_w(w1c_d, "w1c")
            w1a_s = load_w(w1a_d, "w1a")
            w1b_s = load_w(w1b_d, "w1b")
            w1m_s = load_w(w1m_d, "w1m")

            w2m1 = cp.tile([H1, 1], F16, tag="w2m1")
            nc.sync.dma_start(out=w2m1[:], in_=w2m_d[0:H1, :])
            w2m2 = cp.tile([H2, 1], F16, tag="w2m2")
            nc.sync.dma_start(out=w2m2[:], in_=w2m_d[H1:H, :])
            w2p1 = cp.tile([H1, 1], F16, tag="w2p1")
            nc.sync.dma_start(out=w2p1[:], in_=w2p1_d[:])
            w2p2 = cp.tile([33, 1], F16, tag="w2p2")
            nc.sync.dma_start(out=w2p2[:], in_=w2p2_d[:])
            idn = cp.tile([128, 128], F16, tag="idn")
            nc.sync.dma_start(out=idn[:], in_=idn_d[:])
            b1m_c = []
            b1p_c = []
            for h, (ho, hn) in enumerate(hsl):
                bm = cp.tile([hn, 1], F32, tag=f"b1m{h}")
                nc.sync.dma_start(out=bm[:], in_=b1m_d[ho:ho + hn, :])
                b1m_c.append(bm)
                bp = cp.tile([hn, 1], F32, tag=f"b1p{h}")
                nc.sync.dma_start(out=bp[:], in_=b1p_d[ho:ho + hn, :])
                b1p_c.append(bp)
            mm_sb = []
            ma_sb = []
            for mb in range(2):
                mm = cp.tile([128, K + 1], F32, tag=f"mm{mb}")
                nc.sync.dma_start(out=mm[:], in_=mmul_d[128 * mb:128 * (mb + 1), :])
                mm_sb.append(mm)
                ma = cp.tile([128, K + 1], F32, tag=f"ma{mb}")
                nc.sync.dma_start(out=ma[:], in_=madd_d[128 * mb:128 * (mb + 1), :])
                ma_sb.append(ma)

            # activation stores for the deferred w2p phase
            a1all = cp.tile([H1, KM], F16, tag="a1all")
            a2all = cp.tile([33, KM], F16, tag="a2all")
            nc.vector.memset(a2all[:], 0.0)

            # ---- mention score MLP over the full window ----
            ment_act = []
            for h, (ho, hn) in enumerate(hsl):
                psm = pp_pre.tile([hn, W], F32, tag="pre_ps")
                for t in range(NDT):
                    nc.tensor.matmul(psm[:], w1m_s[t][:, ho:ho + hn], xts[t][:],
                                     start=(t == 0), stop=(t == NDT - 1))
                ma = cp.tile([hn, W], F16, tag=f"mact{h}")
                nc.scalar.activation(ma[:], psm[:], AF.Relu, bias=b1m_c[h][:])
                ment_act.append(ma)
            psme = pp_pre.tile([1, W], F32, tag="pre_ps")
            nc.tensor.matmul(psme[:], w2m1[:], ment_act[0][:], start=True, stop=False)
            nc.tensor.matmul(psme[:], w2m2[:], ment_act[1][:], start=False, stop=True)
            ment_row = cp.tile([1, W], F16, tag="mentrow")
            nc.scalar.copy(ment_row[:], psme[:])

            # ment_j carrier row: a2all[32, (delta-1)*MC + m] = ment[m - delta]
            nc.scalar.copy(
                _ap3(a2all[:], 32, 1, 0, [(MC, K), (1, MC)]),
                _ap3(ment_row[:], 0, 1, HB - 1, [(-1, K), (1, MC)]))

            # ment as per-partition columns for the owned 2x128 mention blocks
            ment_col = []
            for mb in range(2):
                pst = pp_pre.tile([128, 1], F16, tag="pre_ps")
                nc.tensor.transpose(pst[:], ment_row[0:1, HB + 128 * mb:HB + 128 * (mb + 1)],
                                    idn[0:1, 0:1])
                mc = cp.tile([128, 1], F32, tag=f"mcol{mb}")
                nc.scalar.copy(mc[:], pst[:])
                ment_col.append(mc)

            # ---- Ya (owned window, + b1p) and Yb (full window) ----
            ya = []
            yb = []
            for h, (ho, hn) in enumerate(hsl):
                psya = pp_pre.tile([hn, MC], F32, tag="pre_ps")
                for t in range(NDT):
                    nc.tensor.matmul(psya[:], w1a_s[t][:, ho:ho + hn], xts[t][:, HB:W],
                                     start=(t == 0), stop=(t == NDT - 1))
                y = cp.tile([hn, MC], F16, tag=f"ya{h}")
                nc.scalar.activation(y[:], psya[:], AF.Identity, bias=b1p_c[h][:])
                ya.append(y)
                psyb = pp_pre.tile([hn, W], F32, tag="pre_ps")
                for t in range(NDT):
                    nc.tensor.matmul(psyb[:], w1b_s[t][:, ho:ho + hn], xts[t][:],
                                     start=(t == 0), stop=(t == NDT - 1))
                y = cp.tile([hn, W], F16, tag=f"yb{h}")
                nc.scalar.copy(y[:], psyb[:])
                yb.append(y)

            # ---- banded pairwise loop ----
            GW = G * MC
            for pg in range((K + GP - 1) // GP):
                p0 = 1 + GP * pg              # first delta of batch
                gp = min(GP, K + 1 - p0)      # deltas in batch

                # products P[d, j, m] = X^T[d, m] * X^T[d, m - (p0+j)]
                pts = []
                for t in range(NDT):
                    pt = wp.tile([128, GP * MC], F16, tag=f"p{t}")
                    nc.vector.tensor_tensor(
                        _ap3(pt[:], 0, 128, 0, [(MC, gp), (1, MC)]),
                        _ap3(xts[t][:], 0, 128, HB, [(0, gp), (1, MC)]),
                        _ap3(xts[t][:], 0, 128, HB - p0, [(-1, gp), (1, MC)]),
                        mybir.AluOpType.mult)
                    pts.append(pt)
                # C[h, j, m] = Ya'[h, m] + Yb[h, m - (p0+j)]
                c1 = wp.tile([H1, GP * MC], F16, tag="c1")
                nc.vector.tensor_tensor(
                    _ap3(c1[:], 0, H1, 0, [(MC, gp), (1, MC)]),
                    _ap3(ya[0][:], 0, H1, 0, [(0, gp), (1, MC)]),
                    _ap3(yb[0][:], 0, H1, HB - p0, [(-1, gp), (1, MC)]),
                    mybir.AluOpType.add)
                c2 = wp.tile([H2, GP * MC], F16, tag="c2")
                nc.vector.tensor_tensor(
                    _ap3(c2[:], 0, H2, 0, [(MC, gp), (1, MC)]),
                    _ap3(ya[1][:], 0, H2, 0, [(0, gp), (1, MC)]),
                    _ap3(yb[1][:], 0, H2, HB - p0, [(-1, gp), (1, MC)]),
                    mybir.AluOpType.add)

                for r in range(gp // G):
                    d0 = p0 + G * r
                    co = G * r * MC            # column offset in batch tiles
                    ko = (d0 - 1) * MC         # column offset in a1all/a2all

                    ps1 = pp_a1.tile([H1, GW], F32, tag="a1")
                    for t in range(NDT):
                        nc.tensor.matmul(ps1[:], w1c_s[t][:, 0:H1],
                                         pts[t][:, co:co + GW],
                                         start=(t == 0), stop=False)
                    nc.tensor.matmul(ps1[:], idn[0:H1, 0:H1], c1[:, co:co + GW],
                                     start=False, stop=True)
                    ps2 = pp_a2.tile([H2, GW], F32, tag="a2")
                    for t in range(NDT):
                        nc.tensor.matmul(ps2[:], w1c_s[t][:, H1:H],
                                         pts[t][:, co:co + GW],
                                         start=(t == 0), stop=False)
                    nc.tensor.matmul(ps2[:], idn[0:H2, 0:H2], c2[:, co:co + GW],
                                     start=False, stop=True)

                    # relu evacuation into the big activation stores
                    nc.scalar.activation(a1all[:, ko:ko + GW], ps1[:], AF.Relu)
                    nc.scalar.activation(a2all[0:H2, ko:ko + GW], ps2[:], AF.Relu)

            # ---- deferred pair phase: pair = w2p . A (+ ment_j carrier) ----
            pair_flat = cp.tile([1, KM], F16, tag="pairflat")
            for r in range(NR):
                co = G * r * MC
                d0 = 1 + G * r
                psp = pp_pair.tile([1, GW], F32, tag="pair")
                nc.tensor.matmul(psp[:], w2p1[:], a1all[:, co:co + GW],
                                 start=True, stop=False)
                nc.tensor.matmul(psp[:], w2p2[:], a2all[:, co:co + GW],
                                 start=False, stop=True)
                for j in range(G):
                    k = K - (d0 + j)
                    nc.scalar.copy(pair_flat[0:1, MC * k:MC * (k + 1)],
                                   psp[0:1, MC * j:MC * (j + 1)])

            # ---- respread (k-major) to rows, transpose, mask, store ----
            pairK = cp.tile([K, MC], F16, tag="pairK")
            nc.sync.dma_start(
                out=pairK[:],
                in_=_ap3(pair_flat[:], 0, 1, 0, [(MC, K), (1, MC)]))
            for mb in range(2):
                pst = pp_pre.tile([128, K], F16, tag="pre_ps")
                nc.tensor.transpose(pst[:], pairK[:, 128 * mb:128 * (mb + 1)],
                                    idn[0:K, 0:K])
                sc = wp.tile([128, K + 1], F32, tag=f"sc{mb}")
                nc.vector.memset(sc[:], 0.0)
                nc.scalar.activation(sc[:, 0:K], pst[:], AF.Identity,
                                     bias=ment_col[mb][:])
                nc.vector.tensor_mul(sc[:], sc[:], mm_sb[mb][:])
                nc.vector.tensor_add(sc[:], sc[:], ma_sb[mb][:])
                nc.sync.dma_start(out=out_d[128 * mb:128 * (mb + 1), :], in_=sc[:])

    nc.compile()
    return nc


def _prep_inputs(inputs):
    X = np.ascontiguousarray(inputs["mention_reprs"], dtype=np.float32)
    assert X.shape == (M, D)
    w1p = np.asarray(inputs["w1p"], dtype=np.float32)
    W1a, W1b, W1c = w1p[:D], w1p[D:2 * D], w1p[2 * D:]
    f16 = lambda a: np.ascontiguousarray(a, dtype=np.float16)

    def padD(w):  # [D, H] -> [DP, H] fp16
        out = np.zeros((DP, H), dtype=np.float16)
        out[:D] = w.astype(np.float16)
        return out

    xtp = np.zeros((DP, M + HB), dtype=np.float16)
    xtp[:D, HB:] = X.T.astype(np.float16)

    w2p = np.asarray(inputs["w2p"], dtype=np.float32)
    shared = {
        "w1c": padD(W1c),
        "w1a": padD(W1a),
        "w1b": padD(W1b),
        "w1m": padD(np.asarray(inputs["w1m"], dtype=np.float32)),
        "w2m": f16(np.asarray(inputs["w2m"], dtype=np.float32).reshape(H, 1)),
        "w2p1": f16(w2p[:H1].reshape(H1, 1)),
        "w2p2": f16(np.concatenate([w2p[H1:], np.zeros(10, np.float32),
                                    [1.0]]).reshape(33, 1)),
        "idn": np.eye(128, dtype=np.float16),
        "b1mc": np.ascontiguousarray(
            np.asarray(inputs["b1m"], dtype=np.float32).reshape(H, 1)),
        "b1pc": np.ascontiguousarray(
            np.asarray(inputs["b1p"], dtype=np.float32).reshape(H, 1)),
    }

    b2m = float(np.asarray(inputs["b2m"]).reshape(-1)[0])
    b2p = float(np.asarray(inputs["b2p"]).reshape(-1)[0])
    in_maps = []
    for c in range(NCORES):
        r0 = MC * c
        xt_c = np.ascontiguousarray(xtp[:, r0:r0 + W])
        mmul = np.ones((MC, K + 1), dtype=np.float32)
        madd = np.full((MC, K + 1), np.float32(b2p + 2.0 * b2m), dtype=np.float32)
        mmul[:, K] = 0.0
        madd[:, K] = 0.0
        if c == 0:
            for i in range(min(K, MC)):
                mmul[i, :K - i] = 0.0
                madd[i, :K - i] = np.float32(-1e9)
        in_maps.append({"xt": xt_c, "mmul": mmul, "madd": madd, **shared})
    return in_maps


def _get_nc(inputs):
    if "nc" not in _cache:
        _cache["nc"] = _build()
    return _cache["nc"]


def _run(inputs, trace=False):
    assert int(np.asarray(inputs["K"])) == K
    nc = _get_nc(inputs)
    in_maps = _prep_inputs(inputs)
    res = run_bass_kernel_spmd(nc, in_maps, list(range(NCORES)), trace=trace)
    out = np.concatenate([res.results[c]["out"] for c in range(NCORES)], axis=0)
    return out.astype(np.float32), res


def kernel(**inputs) -> np.ndarray:
    out, _ = _run(inputs, trace=False)
    return out


# revision 8
# speedup vs baseline: 1.0769x; 1.0769x over previous
"""CorefScore kernel for 8 Trainium2 NeuronCores.

Shards the mention axis M=2048 across 8 cores (256 mentions each plus a
64-row halo of preceding mentions). Per core, the banded pairwise MLP is
computed as 50 shifted elementwise products X^T * shift(X^T, delta) (DVE,
fp16, batched 8 deltas per op) contracted with W1c on the PE in fp16 with
fp32 PSUM accumulation; the Ya + shift(Yb) term is merged on DVE and added
into PSUM via an identity matmul. ReLU activations are stored in large SBUF
buffers; the w2p contraction runs as a deferred back-to-back matmul phase so
the PE stream never waits on ScalarE mid-round. Masking/dummy column are
applied with host-precomputed mask tensors.
"""

import os
import sys

import numpy as np

for _p in ("/opt/trn_rl_repo", "/opt/pypackages"):
    if os.path.isdir(_p) and _p not in sys.path:
        sys.path.append(_p)

import concourse.bacc as bacc
import concourse.bass as bass
import concourse.mybir as mybir
import concourse.tile as tile
from concourse.ap import AP
from concourse.bass_utils import run_bass_kernel_spmd

F16 = mybir.dt.float16
F32 = mybir.dt.float32
AF = mybir.ActivationFunctionType

M, D, H, K = 2048, 900, 150, 50
NCORES = 8
MC = M // NCORES          # owned mentions per core
HB = 64                   # halo columns (>= K)
W = MC + HB               # X^T window width per core
DP = 1024                 # padded feature dim (8 tiles of 128)
NDT = DP // 128           # number of d tiles
G = 2                     # deltas per PSUM round
NR = K // G               # rounds
GP = 8                    # deltas per product batch
H1, H2 = 128, H - 128     # h split
KM = K * MC

_cache = {}


def _ap3(t_ap, p_lo, p_n, off, dims):
    """3-D free-dim view of a tile AP: partitions [p_lo, p_lo+p_n), free
    offset `off` elements, free dims = [(stride, n), ...]."""
    b = t_ap[p_lo:p_lo + p_n, 0:1]
    pstride = b.ap[0][0]
    return AP(b.tensor, b.offset + off, [[pstride, p_n]] + [list(d) for d in dims])


def _build():
    nc = bacc.Bacc("TRN2", target_bir_lowering=False, debug=False)

    xt_d = nc.dram_tensor("xt", [DP, W], F16, kind="ExternalInput").ap()
    w1c_d = nc.dram_tensor("w1c", [DP, H], F16, kind="ExternalInput").ap()
    w1a_d = nc.dram_tensor("w1a", [DP, H], F16, kind="ExternalInput").ap()
    w1b_d = nc.dram_tensor("w1b", [DP, H], F16, kind="ExternalInput").ap()
    w1m_d = nc.dram_tensor("w1m", [DP, H], F16, kind="ExternalInput").ap()
    w2m_d = nc.dram_tensor("w2m", [H, 1], F16, kind="ExternalInput").ap()
    w2p1_d = nc.dram_tensor("w2p1", [H1, 1], F16, kind="ExternalInput").ap()
    w2p2_d = nc.dram_tensor("w2p2", [33, 1], F16, kind="ExternalInput").ap()
    idn_d = nc.dram_tensor("idn", [128, 128], F16, kind="ExternalInput").ap()
    b1m_d = nc.dram_tensor("b1mc", [H, 1], F32, kind="ExternalInput").ap()
    b1p_d = nc.dram_tensor("b1pc", [H, 1], F32, kind="ExternalInput").ap()
    mmul_d = nc.dram_tensor("mmul", [MC, K + 1], F32, kind="ExternalInput").ap()
    madd_d = nc.dram_tensor("madd", [MC, K + 1], F32, kind="ExternalInput").ap()
    out_d = nc.dram_tensor("out", [MC, K + 1], F32, kind="ExternalOutput").ap()

    hsl = [(0, H1), (H1, H2)]  # (h offset, h size) per h tile

    with tile.TileContext(nc) as tc:
        with (
            tc.tile_pool(name="const", bufs=1) as cp,
            tc.tile_pool(name="work", bufs=2) as wp,
            tc.tile_pool(name="ps_pre", bufs=1, space="PSUM") as pp_pre,
            tc.tile_pool(name="ps_a1", bufs=3, space="PSUM") as pp_a1,
            tc.tile_pool(name="ps_a2", bufs=2, space="PSUM") as pp_a2,
            tc.tile_pool(name="ps_pair", bufs=2, space="PSUM") as pp_pair,
        ):
            # ---- load inputs ----
            xts = []
            for t in range(NDT):
                xt = cp.tile([128, W], F16, tag=f"xt{t}")
                nc.sync.dma_start(out=xt[:], in_=xt_d[128 * t:128 * (t + 1), :])
                xts.append(xt)

            def load_w(dram, name):
                ts = []
                for t in range(NDT):
                    w = cp.tile([128, H], F16, tag=f"{name}{t}")
                    nc.sync.dma_start(out=w[:], in_=dram[128 * t:128 * (t + 1), :])
                    ts.append(w)
                return ts

            w1c_s = load_w(w1c_d, "w1c")
            w1a_s = load_w(w1a_d, "w1a")
            w1b_s = load_w(w1b_d, "w1b")
            w1m_s = load_w(w1m_d, "w1m")

            w2m1 = cp.tile([H1, 1], F16, tag="w2m1")
            nc.sync.dma_start(out=w2m1[:], in_=w2m_d[0:H1, :])
            w2m2 = cp.tile([H2, 1], F16, tag="w2m2")
            nc.sync.dma_start(out=w2m2[:], in_=w2m_d[H1:H, :])
            w2p1 = cp.tile([H1, 1], F16, tag="w2p1")
            nc.sync.dma_start(out=w2p1[:], in_=w2p1_d[:])
            w2p2 = cp.tile([33, 1], F16, tag="w2p2")
            nc.sync.dma_start(out=w2p2[:], in_=w2p2_d[:])
            idn = cp.tile([128, 128], F16, tag="idn")
            nc.sync.dma_start(out=idn[:], in_=idn_d[:])
            b1m_c = []
            b1p_c = []
            for h, (ho, hn) in enumerate(hsl):
                bm = cp.tile([hn, 1], F32, tag=f"b1m{h}")
                nc.sync.dma_start(out=bm[:], in_=b1m_d[ho:ho + hn, :])
                b1m_c.append(bm)
                bp = cp.tile([hn, 1], F32, tag=f"b1p{h}")
                nc.sync.dma_start(out=bp[:], in_=b1p_d[ho:ho + hn, :])
                b1p_c.append(bp)
            mm_sb = []
            ma_sb = []
            for mb in range(2):
                mm = cp.tile([128, K + 1], F32, tag=f"mm{mb}")
                nc.sync.dma_start(out=mm[:], in_=mmul_d[128 * mb:128 * (mb + 1), :])
                mm_sb.append(mm)
                ma = cp.tile([128, K + 1], F32, tag=f"ma{mb}")
                nc.sync.dma_start(out=ma[:], in_=madd_d[128 * mb:128 * (mb + 1), :])
                ma_sb.append(ma)

            # activation stores for the deferred w2p phase
            a1all = cp.tile([H1, KM], F16, tag="a1all")
            a2all = cp.tile([33, KM], F16, tag="a2all")
            nc.vector.memset(a2all[:], 0.0)

            # ---- mention score MLP over the full window ----
            ment_act = []
            for h, (ho, hn) in enumerate(hsl):
                psm = pp_pre.tile([hn, W], F32, tag="pre_ps")
                for t in range(NDT):
                    nc.tensor.matmul(psm[:], w1m_s[t][:, ho:ho + hn], xts[t][:],
                                     start=(t == 0), stop=(t == NDT - 1))
                ma = cp.tile([hn, W], F16, tag=f"mact{h}")
                nc.scalar.activation(ma[:], psm[:], AF.Relu, bias=b1m_c[h][:])
                ment_act.append(ma)
            psme = pp_pre.tile([1, W], F32, tag="pre_ps")
            nc.tensor.matmul(psme[:], w2m1[:], ment_act[0][:], start=True, stop=False)
            nc.tensor.matmul(psme[:], w2m2[:], ment_act[1][:], start=False, stop=True)
            ment_row = cp.tile([1, W], F16, tag="mentrow")
            nc.scalar.copy(ment_row[:], psme[:])

            # ment_j carrier row: a2all[32, (delta-1)*MC + m] = ment[m - delta]
            nc.scalar.copy(
                _ap3(a2all[:], 32, 1, 0, [(MC, K), (1, MC)]),
                _ap3(ment_row[:], 0, 1, HB - 1, [(-1, K), (1, MC)]))

            # ment as per-partition columns for the owned 2x128 mention blocks
            ment_col = []
            for mb in range(2):
                pst = pp_pre.tile([128, 1], F16, tag="pre_ps")
                nc.tensor.transpose(pst[:], ment_row[0:1, HB + 128 * mb:HB + 128 * (mb + 1)],
                                    idn[0:1, 0:1])
                mc = cp.tile([128, 1], F32, tag=f"mcol{mb}")
                nc.scalar.copy(mc[:], pst[:])
                ment_col.append(mc)

            # ---- Ya (owned window, + b1p) and Yb (full window) ----
            ya = []
            yb = []
            for h, (ho, hn) in enumerate(hsl):
                psya = pp_pre.tile([hn, MC], F32, tag="pre_ps")
                for t in range(NDT):
                    nc.tensor.matmul(psya[:], w1a_s[t][:, ho:ho + hn], xts[t][:, HB:W],
                                     start=(t == 0), stop=(t == NDT - 1))
                y = cp.tile([hn, MC], F16, tag=f"ya{h}")
                nc.scalar.activation(y[:], psya[:], AF.Identity, bias=b1p_c[h][:])
                ya.append(y)
                psyb = pp_pre.tile([hn, W], F32, tag="pre_ps")
                for t in range(NDT):
                    nc.tensor.matmul(psyb[:], w1b_s[t][:, ho:ho + hn], xts[t][:],
                                     start=(t == 0), stop=(t == NDT - 1))
                y = cp.tile([hn, W], F16, tag=f"yb{h}")
                nc.scalar.copy(y[:], psyb[:])
                yb.append(y)

            # ---- banded pairwise loop ----
            GW = G * MC
            pair_flat = cp.tile([1, KM], F16, tag="pairflat")
            for pg in range((K + GP - 1) // GP):
                p0 = 1 + GP * pg              # first delta of batch
                gp = min(GP, K + 1 - p0)      # deltas in batch

                # products P[d, j, m] = X^T[d, m] * X^T[d, m - (p0+j)]
                pts = []
                for t in range(NDT):
                    pt = wp.tile([128, GP * MC], F16, tag=f"p{t}")
                    nc.vector.tensor_tensor(
                        _ap3(pt[:], 0, 128, 0, [(MC, gp), (1, MC)]),
                        _ap3(xts[t][:], 0, 128, HB, [(0, gp), (1, MC)]),
                        _ap3(xts[t][:], 0, 128, HB - p0, [(-1, gp), (1, MC)]),
                        mybir.AluOpType.mult)
                    pts.append(pt)
                # C[h, j, m] = Ya'[h, m] + Yb[h, m - (p0+j)]
                c1 = wp.tile([H1, GP * MC], F16, tag="c1")
                nc.vector.tensor_tensor(
                    _ap3(c1[:], 0, H1, 0, [(MC, gp), (1, MC)]),
                    _ap3(ya[0][:], 0, H1, 0, [(0, gp), (1, MC)]),
                    _ap3(yb[0][:], 0, H1, HB - p0, [(-1, gp), (1, MC)]),
                    mybir.AluOpType.add)
                c2 = wp.tile([H2, GP * MC], F16, tag="c2")
                nc.vector.tensor_tensor(
                    _ap3(c2[:], 0, H2, 0, [(MC, gp), (1, MC)]),
                    _ap3(ya[1][:], 0, H2, 0, [(0, gp), (1, MC)]),
                    _ap3(yb[1][:], 0, H2, HB - p0, [(-1, gp), (1, MC)]),
                    mybir.AluOpType.add)

                for r in range(gp // G):
                    d0 = p0 + G * r
                    co = G * r * MC            # column offset in batch tiles
                    ko = (d0 - 1) * MC         # column offset in a1all/a2all

                    ps1 = pp_a1.tile([H1, GW], F32, tag="a1")
                    for t in range(NDT):
                        nc.tensor.matmul(ps1[:], w1c_s[t][:, 0:H1],
                                         pts[t][:, co:co + GW],
                                         start=(t == 0), stop=False)
                    nc.tensor.matmul(ps1[:], idn[0:H1, 0:H1], c1[:, co:co + GW],
                                     start=False, stop=True)
                    ps2 = pp_a2.tile([H2, GW], F32, tag="a2")
                    for t in range(NDT):
                        nc.tensor.matmul(ps2[:], w1c_s[t][:, H1:H],
                                         pts[t][:, co:co + GW],
                                         start=(t == 0), stop=False)
                    nc.tensor.matmul(ps2[:], idn[0:H2, 0:H2], c2[:, co:co + GW],
                                     start=False, stop=True)

                    # relu evacuation into the big activation stores
                    nc.scalar.activation(a1all[:, ko:ko + GW], ps1[:], AF.Relu)
                    nc.scalar.activation(a2all[0:H2, ko:ko + GW], ps2[:], AF.Relu)

                    # pair = w2p . A (+ ment_j carrier row)
                    psp = pp_pair.tile([1, GW], F32, tag="pair")
                    nc.tensor.matmul(psp[:], w2p1[:], a1all[:, ko:ko + GW],
                                     start=True, stop=False)
                    nc.tensor.matmul(psp[:], w2p2[:], a2all[:, ko:ko + GW],
                                     start=False, stop=True)
                    for j in range(G):
                        k = K - (d0 + j)
                        nc.scalar.copy(pair_flat[0:1, MC * k:MC * (k + 1)],
                                       psp[0:1, MC * j:MC * (j + 1)])

            # ---- respread (k-major) to rows, transpose, mask, store ----
            pairK = cp.tile([K, MC], F16, tag="pairK")
            nc.sync.dma_start(
                out=pairK[:],
                in_=_ap3(pair_flat[:], 0, 1, 0, [(MC, K), (1, MC)]))
            for mb in range(2):
                pst = pp_pre.tile([128, K], F16, tag="pre_ps")
                nc.tensor.transpose(pst[:], pairK[:, 128 * mb:128 * (mb + 1)],
                                    idn[0:K, 0:K])
                sc = wp.tile([128, K + 1], F32, tag=f"sc{mb}")
                nc.vector.memset(sc[:], 0.0)
                nc.scalar.activation(sc[:, 0:K], pst[:], AF.Identity,
                                     bias=ment_col[mb][:])
                nc.vector.tensor_mul(sc[:], sc[:], mm_sb[mb][:])
                nc.vector.tensor_add(sc[:], sc[:], ma_sb[mb][:])
                nc.sync.dma_start(out=out_d[128 * mb:128 * (mb + 1), :], in_=sc[:])

    nc.compile()
    return nc


def _prep_inputs(inputs):
    X = np.ascontiguousarray(inputs["mention_reprs"], dtype=np.float32)
    assert X.shape == (M, D)
    w1p = np.asarray(inputs["w1p"], dtype=np.float32)
    W1a, W1b, W1c = w1p[:D], w1p[D:2 * D], w1p[2 * D:]
    f16 = lambda a: np.ascontiguousarray(a, dtype=np.float16)

    def padD(w):  # [D, H] -> [DP, H] fp16
        out = np.zeros((DP, H), dtype=np.float16)
        out[:D] = w.astype(np.float16)
        return out

    xtp = np.zeros((DP, M + HB), dtype=np.float16)
    xtp[:D, HB:] = X.T.astype(np.float16)

    w2p = np.asarray(inputs["w2p"], dtype=np.float32)
    shared = {
        "w1c": padD(W1c),
        "w1a": padD(W1a),
        "w1b": padD(W1b),
        "w1m": padD(np.asarray(inputs["w1m"], dtype=np.float32)),
        "w2m": f16(np.asarray(inputs["w2m"], dtype=np.float32).reshape(H, 1)),
        "w2p1": f16(w2p[:H1].reshape(H1, 1)),
        "w2p2": f16(np.concatenate([w2p[H1:], np.zeros(10, np.float32),
                                    [1.0]]).reshape(33, 1)),
        "idn": np.eye(128, dtype=np.float16),
        "b1mc": np.ascontiguousarray(
            np.asarray(inputs["b1m"], dtype=np.float32).reshape(H, 1)),
        "b1pc": np.ascontiguousarray(
            np.asarray(inputs["b1p"], dtype=np.float32).reshape(H, 1)),
    }

    b2m = float(np.asarray(inputs["b2m"]).reshape(-1)[0])
    b2p = float(np.asarray(inputs["b2p"]).reshape(-1)[0])
    in_maps = []
    for c in range(NCORES):
        r0 = MC * c
        xt_c = np.ascontiguousarray(xtp[:, r0:r0 + W])
        mmul = np.ones((MC, K + 1), dtype=np.float32)
        madd = np.full((MC, K + 1), np.float32(b2p + 2.0 * b2m), dtype=np.float32)
        mmul[:, K] = 0.0
        madd[:, K] = 0.0
        if c == 0:
            for i in range(min(K, MC)):
                mmul[i, :K - i] = 0.0
                madd[i, :K - i] = np.float32(-1e9)
        in_maps.append({"xt": xt_c, "mmul": mmul, "madd": madd, **shared})
    return in_maps


def _get_nc(inputs):
    if "nc" not in _cache:
        _cache["nc"] = _build()
    return _cache["nc"]


def _run(inputs, trace=False):
    assert int(np.asarray(inputs["K"])) == K
    nc = _get_nc(inputs)
    in_maps = _prep_inputs(inputs)
    res = run_bass_kernel_spmd(nc, in_maps, list(range(NCORES)), trace=trace)
    out = np.concatenate([res.results[c]["out"] for c in range(NCORES)], axis=0)
    return out.astype(np.float32), res


def kernel(**inputs) -> np.ndarray:
    out, _ = _run(inputs, trace=False)
    return out


# revision 9
# speedup vs baseline: 1.1347x; 1.0537x over previous
"""CorefScore kernel for 8 Trainium2 NeuronCores.

Shards the mention axis M=2048 across 8 cores (256 mentions each plus a
64-row halo of preceding mentions). Per core, the banded pairwise MLP is
computed as 50 shifted elementwise products X^T * shift(X^T, delta) (DVE,
fp16, batched 8 deltas per op) contracted with W1c on the PE in fp16 with
fp32 PSUM accumulation; the Ya + shift(Yb) term is merged on DVE and added
into PSUM via an identity matmul. ReLU activations are stored in large SBUF
buffers; the w2p contraction runs as a deferred back-to-back matmul phase so
the PE stream never waits on ScalarE mid-round. Masking/dummy column are
applied with host-precomputed mask tensors.
"""

import os
import sys

import numpy as np

for _p in ("/opt/trn_rl_repo", "/opt/pypackages"):
    if os.path.isdir(_p) and _p not in sys.path:
        sys.path.append(_p)

import concourse.bacc as bacc
import concourse.bass as bass
import concourse.mybir as mybir
import concourse.tile as tile
from concourse.ap import AP
from concourse.bass_utils import run_bass_kernel_spmd

F16 = mybir.dt.float16
F32 = mybir.dt.float32
AF = mybir.ActivationFunctionType

M, D, H, K = 2048, 900, 150, 50
NCORES = 8
MC = M // NCORES          # owned mentions per core
HB = 64                   # halo columns (>= K)
W = MC + HB               # X^T window width per core
DP = 1024                 # padded feature dim (8 tiles of 128)
NDT = DP // 128           # number of d tiles
G = 2                     # deltas per PSUM round
NR = K // G               # rounds
GP = 8                    # deltas per product batch
H1, H2 = 128, H - 128     # h split
KM = K * MC

_cache = {}


def _ap3(t_ap, p_lo, p_n, off, dims):
    """3-D free-dim view of a tile AP: partitions [p_lo, p_lo+p_n), free
    offset `off` elements, free dims = [(stride, n), ...]."""
    b = t_ap[p_lo:p_lo + p_n, 0:1]
    pstride = b.ap[0][0]
    return AP(b.tensor, b.offset + off, [[pstride, p_n]] + [list(d) for d in dims])


def _build():
    nc = bacc.Bacc("TRN2", target_bir_lowering=False, debug=False)

    xt_d = nc.dram_tensor("xt", [DP, W], F16, kind="ExternalInput").ap()
    w1c_d = nc.dram_tensor("w1c", [DP, H], F16, kind="ExternalInput").ap()
    w1a_d = nc.dram_tensor("w1a", [DP, H], F16, kind="ExternalInput").ap()
    w1b_d = nc.dram_tensor("w1b", [DP, H], F16, kind="ExternalInput").ap()
    w1m_d = nc.dram_tensor("w1m", [DP, H], F16, kind="ExternalInput").ap()
    w2m_d = nc.dram_tensor("w2m", [H, 1], F16, kind="ExternalInput").ap()
    w2p1_d = nc.dram_tensor("w2p1", [H1, 1], F16, kind="ExternalInput").ap()
    w2p2_d = nc.dram_tensor("w2p2", [33, 1], F16, kind="ExternalInput").ap()
    idn_d = nc.dram_tensor("idn", [128, 128], F16, kind="ExternalInput").ap()
    b1m_d = nc.dram_tensor("b1mc", [H, 1], F32, kind="ExternalInput").ap()
    b1p_d = nc.dram_tensor("b1pc", [H, 1], F32, kind="ExternalInput").ap()
    mmul_d = nc.dram_tensor("mmul", [MC, K + 1], F32, kind="ExternalInput").ap()
    madd_d = nc.dram_tensor("madd", [MC, K + 1], F32, kind="ExternalInput").ap()
    out_d = nc.dram_tensor("out", [MC, K + 1], F32, kind="ExternalOutput").ap()

    hsl = [(0, H1), (H1, H2)]  # (h offset, h size) per h tile

    with tile.TileContext(nc) as tc:
        with (
            tc.tile_pool(name="const", bufs=1) as cp,
            tc.tile_pool(name="work", bufs=2) as wp,
            tc.tile_pool(name="ps_pre", bufs=1, space="PSUM") as pp_pre,
            tc.tile_pool(name="ps_a1", bufs=3, space="PSUM") as pp_a1,
            tc.tile_pool(name="ps_a2", bufs=2, space="PSUM") as pp_a2,
            tc.tile_pool(name="ps_pair", bufs=2, space="PSUM") as pp_pair,
        ):
            # ---- load inputs ----
            xts = []
            for t in range(NDT):
                xt = cp.tile([128, W], F16, tag=f"xt{t}")
                nc.sync.dma_start(out=xt[:], in_=xt_d[128 * t:128 * (t + 1), :])
                xts.append(xt)

            def load_w(dram, name):
                ts = []
                for t in range(NDT):
                    w = cp.tile([128, H], F16, tag=f"{name}{t}")
                    nc.sync.dma_start(out=w[:], in_=dram[128 * t:128 * (t + 1), :])
                    ts.append(w)
                return ts

            w1c_s = load_w(w1c_d, "w1c")
            w1a_s = load_w(w1a_d, "w1a")
            w1b_s = load_w(w1b_d, "w1b")
            w1m_s = load_w(w1m_d, "w1m")

            w2m1 = cp.tile([H1, 1], F16, tag="w2m1")
            nc.sync.dma_start(out=w2m1[:], in_=w2m_d[0:H1, :])
            w2m2 = cp.tile([H2, 1], F16, tag="w2m2")
            nc.sync.dma_start(out=w2m2[:], in_=w2m_d[H1:H, :])
            w2p1 = cp.tile([H1, 1], F16, tag="w2p1")
            nc.sync.dma_start(out=w2p1[:], in_=w2p1_d[:])
            w2p2 = cp.tile([33, 1], F16, tag="w2p2")
            nc.sync.dma_start(out=w2p2[:], in_=w2p2_d[:])
            idn = cp.tile([128, 128], F16, tag="idn")
            nc.sync.dma_start(out=idn[:], in_=idn_d[:])
            b1m_c = []
            b1p_c = []
            for h, (ho, hn) in enumerate(hsl):
                bm = cp.tile([hn, 1], F32, tag=f"b1m{h}")
                nc.sync.dma_start(out=bm[:], in_=b1m_d[ho:ho + hn, :])
                b1m_c.append(bm)
                bp = cp.tile([hn, 1], F32, tag=f"b1p{h}")
                nc.sync.dma_start(out=bp[:], in_=b1p_d[ho:ho + hn, :])
                b1p_c.append(bp)
            mm_sb = []
            ma_sb = []
            for mb in range(2):
                mm = cp.tile([128, K + 1], F32, tag=f"mm{mb}")
                nc.sync.dma_start(out=mm[:], in_=mmul_d[128 * mb:128 * (mb + 1), :])
                mm_sb.append(mm)
                ma = cp.tile([128, K + 1], F32, tag=f"ma{mb}")
                nc.sync.dma_start(out=ma[:], in_=madd_d[128 * mb:128 * (mb + 1), :])
                ma_sb.append(ma)

            # ---- mention score MLP over the full window ----
            ment_act = []
            for h, (ho, hn) in enumerate(hsl):
                psm = pp_pre.tile([hn, W], F32, tag="pre_ps")
                for t in range(NDT):
                    nc.tensor.matmul(psm[:], w1m_s[t][:, ho:ho + hn], xts[t][:],
                                     start=(t == 0), stop=(t == NDT - 1))
                ma = cp.tile([hn, W], F16, tag=f"mact{h}")
                nc.scalar.activation(ma[:], psm[:], AF.Relu, bias=b1m_c[h][:])
                ment_act.append(ma)
            psme = pp_pre.tile([1, W], F32, tag="pre_ps")
            nc.tensor.matmul(psme[:], w2m1[:], ment_act[0][:], start=True, stop=False)
            nc.tensor.matmul(psme[:], w2m2[:], ment_act[1][:], start=False, stop=True)
            ment_row = cp.tile([1, W], F16, tag="mentrow")
            nc.scalar.copy(ment_row[:], psme[:])

            # ment as per-partition columns for the owned 2x128 mention blocks
            ment_col = []
            for mb in range(2):
                pst = pp_pre.tile([128, 1], F16, tag="pre_ps")
                nc.tensor.transpose(pst[:], ment_row[0:1, HB + 128 * mb:HB + 128 * (mb + 1)],
                                    idn[0:1, 0:1])
                mc = cp.tile([128, 1], F32, tag=f"mcol{mb}")
                nc.scalar.copy(mc[:], pst[:])
                ment_col.append(mc)

            # ---- Ya (owned window, + b1p) and Yb (full window) ----
            ya = []
            yb = []
            for h, (ho, hn) in enumerate(hsl):
                psya = pp_pre.tile([hn, MC], F32, tag="pre_ps")
                for t in range(NDT):
                    nc.tensor.matmul(psya[:], w1a_s[t][:, ho:ho + hn], xts[t][:, HB:W],
                                     start=(t == 0), stop=(t == NDT - 1))
                y = cp.tile([hn, MC], F16, tag=f"ya{h}")
                nc.scalar.activation(y[:], psya[:], AF.Identity, bias=b1p_c[h][:])
                ya.append(y)
                psyb = pp_pre.tile([hn, W], F32, tag="pre_ps")
                for t in range(NDT):
                    nc.tensor.matmul(psyb[:], w1b_s[t][:, ho:ho + hn], xts[t][:],
                                     start=(t == 0), stop=(t == NDT - 1))
                y = cp.tile([hn, W], F16, tag=f"yb{h}")
                nc.scalar.copy(y[:], psyb[:])
                yb.append(y)

            # ---- banded pairwise loop: G deltas per round ----
            GW = G * MC
            pair_flat = cp.tile([1, KM], F16, tag="pairflat")
            a2x_bufs = []
            for i in range(2):
                ab = cp.tile([33, GW], F16, tag=f"a2x{i}")
                nc.vector.memset(ab[:], 0.0)
                a2x_bufs.append(ab)
            for r in range(NR):
                d0 = 1 + G * r  # deltas d0 .. d0+G-1

                # products P[d, j, m] = X^T[d, m] * X^T[d, m - (d0+j)]
                pts = []
                for t in range(NDT):
                    pt = wp.tile([128, GW], F16, tag=f"p{t}")
                    nc.vector.tensor_tensor(
                        _ap3(pt[:], 0, 128, 0, [(MC, G), (1, MC)]),
                        _ap3(xts[t][:], 0, 128, HB, [(0, G), (1, MC)]),
                        _ap3(xts[t][:], 0, 128, HB - d0, [(-1, G), (1, MC)]),
                        mybir.AluOpType.mult)
                    pts.append(pt)

                # C[h, j, m] = Ya'[h, m] + Yb[h, m - (d0+j)]
                c1 = wp.tile([H1, GW], F16, tag="c1")
                nc.vector.tensor_tensor(
                    _ap3(c1[:], 0, H1, 0, [(MC, G), (1, MC)]),
                    _ap3(ya[0][:], 0, H1, 0, [(0, G), (1, MC)]),
                    _ap3(yb[0][:], 0, H1, HB - d0, [(-1, G), (1, MC)]),
                    mybir.AluOpType.add)
                c2 = wp.tile([H2, GW], F16, tag="c2")
                nc.vector.tensor_tensor(
                    _ap3(c2[:], 0, H2, 0, [(MC, G), (1, MC)]),
                    _ap3(ya[1][:], 0, H2, 0, [(0, G), (1, MC)]),
                    _ap3(yb[1][:], 0, H2, HB - d0, [(-1, G), (1, MC)]),
                    mybir.AluOpType.add)

                # A = P @ W1c + C, per h tile, PSUM-accumulated
                ps1 = pp_a1.tile([H1, GW], F32, tag="a1")
                for t in range(NDT):
                    nc.tensor.matmul(ps1[:], w1c_s[t][:, 0:H1], pts[t][:],
                                     start=(t == 0), stop=False)
                nc.tensor.matmul(ps1[:], idn[0:H1, 0:H1], c1[:], start=False, stop=True)
                ps2 = pp_a2.tile([H2, GW], F32, tag="a2")
                for t in range(NDT):
                    nc.tensor.matmul(ps2[:], w1c_s[t][:, H1:H], pts[t][:],
                                     start=(t == 0), stop=False)
                nc.tensor.matmul(ps2[:], idn[0:H2, 0:H2], c2[:], start=False, stop=True)

                # relu evacuation (b1p already inside C via Ya)
                a1 = wp.tile([H1, GW], F16, tag="a1sb")
                nc.scalar.activation(a1[:], ps1[:], AF.Relu)
                a2x = a2x_bufs[r % 2]
                nc.scalar.activation(a2x[0:H2, :], ps2[:], AF.Relu)
                # ment_j carrier row (at partition 32; w2p2 rows 22..31 are 0)
                nc.scalar.copy(
                    _ap3(a2x[:], 32, 1, 0, [(MC, G), (1, MC)]),
                    _ap3(ment_row[:], 0, 1, HB - d0, [(-1, G), (1, MC)]))

                # pair = w2p . A  (+ ment_j via carrier)
                psp = pp_pair.tile([1, GW], F32, tag="pair")
                nc.tensor.matmul(psp[:], w2p1[:], a1[:], start=True, stop=False)
                nc.tensor.matmul(psp[:], w2p2[:], a2x[:], start=False, stop=True)
                for j in range(G):
                    k = K - (d0 + j)
                    nc.scalar.copy(pair_flat[0:1, MC * k:MC * (k + 1)],
                                   psp[0:1, MC * j:MC * (j + 1)])

            # ---- respread (k-major) to rows, transpose, mask, store ----
            pairK = cp.tile([K, MC], F16, tag="pairK")
            nc.sync.dma_start(
                out=pairK[:],
                in_=_ap3(pair_flat[:], 0, 1, 0, [(MC, K), (1, MC)]))
            for mb in range(2):
                pst = pp_pre.tile([128, K], F16, tag="pre_ps")
                nc.tensor.transpose(pst[:], pairK[:, 128 * mb:128 * (mb + 1)],
                                    idn[0:K, 0:K])
                sc = wp.tile([128, K + 1], F32, tag=f"sc{mb}")
                nc.vector.memset(sc[:], 0.0)
                nc.scalar.activation(sc[:, 0:K], pst[:], AF.Identity,
                                     bias=ment_col[mb][:])
                nc.vector.tensor_mul(sc[:], sc[:], mm_sb[mb][:])
                nc.vector.tensor_add(sc[:], sc[:], ma_sb[mb][:])
                nc.sync.dma_start(out=out_d[128 * mb:128 * (mb + 1), :], in_=sc[:])

    nc.compile()
    return nc


def _prep_inputs(inputs):
    X = np.ascontiguousarray(inputs["mention_reprs"], dtype=np.float32)
    assert X.shape == (M, D)
    w1p = np.asarray(inputs["w1p"], dtype=np.float32)
    W1a, W1b, W1c = w1p[:D], w1p[D:2 * D], w1p[2 * D:]
    f16 = lambda a: np.ascontiguousarray(a, dtype=np.float16)

    def padD(w):  # [D, H] -> [DP, H] fp16
        out = np.zeros((DP, H), dtype=np.float16)
        out[:D] = w.astype(np.float16)
        return out

    xtp = np.zeros((DP, M + HB), dtype=np.float16)
    xtp[:D, HB:] = X.T.astype(np.float16)

    w2p = np.asarray(inputs["w2p"], dtype=np.float32)
    shared = {
        "w1c": padD(W1c),
        "w1a": padD(W1a),
        "w1b": padD(W1b),
        "w1m": padD(np.asarray(inputs["w1m"], dtype=np.float32)),
        "w2m": f16(np.asarray(inputs["w2m"], dtype=np.float32).reshape(H, 1)),
        "w2p1": f16(w2p[:H1].reshape(H1, 1)),
        "w2p2": f16(np.concatenate([w2p[H1:], np.zeros(10, np.float32),
                                    [1.0]]).reshape(33, 1)),
        "idn": np.eye(128, dtype=np.float16),
        "b1mc": np.ascontiguousarray(
            np.asarray(inputs["b1m"], dtype=np.float32).reshape(H, 1)),
        "b1pc": np.ascontiguousarray(
            np.asarray(inputs["b1p"], dtype=np.float32).reshape(H, 1)),
    }

    b2m = float(np.asarray(inputs["b2m"]).reshape(-1)[0])
    b2p = float(np.asarray(inputs["b2p"]).reshape(-1)[0])
    in_maps = []
    for c in range(NCORES):
        r0 = MC * c
        xt_c = np.ascontiguousarray(xtp[:, r0:r0 + W])
        mmul = np.ones((MC, K + 1), dtype=np.float32)
        madd = np.full((MC, K + 1), np.float32(b2p + 2.0 * b2m), dtype=np.float32)
        mmul[:, K] = 0.0
        madd[:, K] = 0.0
        if c == 0:
            for i in range(min(K, MC)):
                mmul[i, :K - i] = 0.0
                madd[i, :K - i] = np.float32(-1e9)
        in_maps.append({"xt": xt_c, "mmul": mmul, "madd": madd, **shared})
    return in_maps


def _get_nc(inputs):
    if "nc" not in _cache:
        _cache["nc"] = _build()
    return _cache["nc"]


def _run(inputs, trace=False):
    assert int(np.asarray(inputs["K"])) == K
    nc = _get_nc(inputs)
    in_maps = _prep_inputs(inputs)
    res = run_bass_kernel_spmd(nc, in_maps, list(range(NCORES)), trace=trace)
    out = np.concatenate([res.results[c]["out"] for c in range(NCORES)], axis=0)
    return out.astype(np.float32), res


def kernel(**inputs) -> np.ndarray:
    out, _ = _run(inputs, trace=False)
    return out


# revision 10
# speedup vs baseline: 1.1589x; 1.0213x over previous
"""CorefScore kernel for 8 Trainium2 NeuronCores.

Shards the mention axis M=2048 across 8 cores (256 mentions each plus a
64-row halo of preceding mentions). Per core, the banded pairwise MLP is
computed as 50 shifted elementwise products X^T * shift(X^T, delta) (DVE,
fp16, batched 8 deltas per op) contracted with W1c on the PE in fp16 with
fp32 PSUM accumulation; the Ya + shift(Yb) term is merged on DVE and added
into PSUM via an identity matmul. ReLU activations are stored in large SBUF
buffers; the w2p contraction runs as a deferred back-to-back matmul phase so
the PE stream never waits on ScalarE mid-round. Masking/dummy column are
applied with host-precomputed mask tensors.
"""

import os
import sys

import numpy as np

for _p in ("/opt/trn_rl_repo", "/opt/pypackages"):
    if os.path.isdir(_p) and _p not in sys.path:
        sys.path.append(_p)

import concourse.bacc as bacc
import concourse.bass as bass
import concourse.mybir as mybir
import concourse.tile as tile
from concourse.ap import AP
from concourse.bass_utils import run_bass_kernel_spmd

F16 = mybir.dt.float16
F32 = mybir.dt.float32
AF = mybir.ActivationFunctionType

M, D, H, K = 2048, 900, 150, 50
NCORES = 8
MC = M // NCORES          # owned mentions per core
HB = 64                   # halo columns (>= K)
W = MC + HB               # X^T window width per core
DP = 1024                 # padded feature dim (8 tiles of 128)
NDT = DP // 128           # number of d tiles
G = 2                     # deltas per PSUM round
NR = K // G               # rounds
GP = 8                    # deltas per product batch
H1, H2 = 128, H - 128     # h split
KM = K * MC

_cache = {}


def _ap3(t_ap, p_lo, p_n, off, dims):
    """3-D free-dim view of a tile AP: partitions [p_lo, p_lo+p_n), free
    offset `off` elements, free dims = [(stride, n), ...]."""
    b = t_ap[p_lo:p_lo + p_n, 0:1]
    pstride = b.ap[0][0]
    return AP(b.tensor, b.offset + off, [[pstride, p_n]] + [list(d) for d in dims])


def _build():
    nc = bacc.Bacc("TRN2", target_bir_lowering=False, debug=False)

    xt_d = nc.dram_tensor("xt", [DP, W], F16, kind="ExternalInput").ap()
    w1c_d = nc.dram_tensor("w1c", [DP, H], F16, kind="ExternalInput").ap()
    w1a_d = nc.dram_tensor("w1a", [DP, H], F16, kind="ExternalInput").ap()
    w1b_d = nc.dram_tensor("w1b", [DP, H], F16, kind="ExternalInput").ap()
    w1m_d = nc.dram_tensor("w1m", [DP, H], F16, kind="ExternalInput").ap()
    w2m_d = nc.dram_tensor("w2m", [H, 1], F16, kind="ExternalInput").ap()
    w2p1_d = nc.dram_tensor("w2p1", [H1, 1], F16, kind="ExternalInput").ap()
    w2p2_d = nc.dram_tensor("w2p2", [33, 1], F16, kind="ExternalInput").ap()
    idn_d = nc.dram_tensor("idn", [128, 128], F16, kind="ExternalInput").ap()
    b1m_d = nc.dram_tensor("b1mc", [H, 1], F32, kind="ExternalInput").ap()
    b1p_d = nc.dram_tensor("b1pc", [H, 1], F32, kind="ExternalInput").ap()
    mmul_d = nc.dram_tensor("mmul", [MC, K + 1], F32, kind="ExternalInput").ap()
    madd_d = nc.dram_tensor("madd", [MC, K + 1], F32, kind="ExternalInput").ap()
    out_d = nc.dram_tensor("out", [MC, K + 1], F32, kind="ExternalOutput").ap()

    hsl = [(0, H1), (H1, H2)]  # (h offset, h size) per h tile

    with tile.TileContext(nc) as tc:
        with (
            tc.tile_pool(name="const", bufs=1) as cp,
            tc.tile_pool(name="work", bufs=2) as wp,
            tc.tile_pool(name="ps_pre", bufs=2, space="PSUM") as pp_pre,
            tc.tile_pool(name="ps_a1", bufs=2, space="PSUM") as pp_a1,
            tc.tile_pool(name="ps_a2", bufs=2, space="PSUM") as pp_a2,
            tc.tile_pool(name="ps_pair", bufs=2, space="PSUM") as pp_pair,
        ):
            # ---- load inputs ----
            xts = []
            for t in range(NDT):
                xt = cp.tile([128, W], F16, tag=f"xt{t}")
                nc.sync.dma_start(out=xt[:], in_=xt_d[128 * t:128 * (t + 1), :])
                xts.append(xt)

            def load_w(dram, name):
                ts = []
                for t in range(NDT):
                    w = cp.tile([128, H], F16, tag=f"{name}{t}")
                    nc.sync.dma_start(out=w[:], in_=dram[128 * t:128 * (t + 1), :])
                    ts.append(w)
                return ts

            w1c_s = load_w(w1c_d, "w1c")
            w1a_s = load_w(w1a_d, "w1a")
            w1b_s = load_w(w1b_d, "w1b")
            w1m_s = load_w(w1m_d, "w1m")

            w2m1 = cp.tile([H1, 1], F16, tag="w2m1")
            nc.sync.dma_start(out=w2m1[:], in_=w2m_d[0:H1, :])
            w2m2 = cp.tile([H2, 1], F16, tag="w2m2")
            nc.sync.dma_start(out=w2m2[:], in_=w2m_d[H1:H, :])
            w2p1 = cp.tile([H1, 1], F16, tag="w2p1")
            nc.sync.dma_start(out=w2p1[:], in_=w2p1_d[:])
            w2p2 = cp.tile([33, 1], F16, tag="w2p2")
            nc.sync.dma_start(out=w2p2[:], in_=w2p2_d[:])
            idn = cp.tile([128, 128], F16, tag="idn")
            nc.sync.dma_start(out=idn[:], in_=idn_d[:])
            b1m_c = []
            b1p_c = []
            for h, (ho, hn) in enumerate(hsl):
                bm = cp.tile([hn, 1], F32, tag=f"b1m{h}")
                nc.sync.dma_start(out=bm[:], in_=b1m_d[ho:ho + hn, :])
                b1m_c.append(bm)
                bp = cp.tile([hn, 1], F32, tag=f"b1p{h}")
                nc.sync.dma_start(out=bp[:], in_=b1p_d[ho:ho + hn, :])
                b1p_c.append(bp)
            mm_sb = []
            ma_sb = []
            for mb in range(2):
                mm = cp.tile([128, K + 1], F32, tag=f"mm{mb}")
                nc.sync.dma_start(out=mm[:], in_=mmul_d[128 * mb:128 * (mb + 1), :])
                mm_sb.append(mm)
                ma = cp.tile([128, K + 1], F32, tag=f"ma{mb}")
                nc.sync.dma_start(out=ma[:], in_=madd_d[128 * mb:128 * (mb + 1), :])
                ma_sb.append(ma)

            # ---- mention score MLP over the full window ----
            ment_act = []
            for h, (ho, hn) in enumerate(hsl):
                psm = pp_pre.tile([hn, W], F32, tag="pre_ps")
                for t in range(NDT):
                    nc.tensor.matmul(psm[:], w1m_s[t][:, ho:ho + hn], xts[t][:],
                                     start=(t == 0), stop=(t == NDT - 1))
                ma = cp.tile([hn, W], F16, tag=f"mact{h}")
                nc.scalar.activation(ma[:], psm[:], AF.Relu, bias=b1m_c[h][:])
                ment_act.append(ma)
            psme = pp_pre.tile([1, W], F32, tag="pre_ps")
            nc.tensor.matmul(psme[:], w2m1[:], ment_act[0][:], start=True, stop=False)
            nc.tensor.matmul(psme[:], w2m2[:], ment_act[1][:], start=False, stop=True)
            ment_row = cp.tile([1, W], F16, tag="mentrow")
            nc.scalar.copy(ment_row[:], psme[:])

            # ment as per-partition columns for the owned 2x128 mention blocks
            ment_col = []
            for mb in range(2):
                pst = pp_pre.tile([128, 1], F16, tag="pre_ps")
                nc.tensor.transpose(pst[:], ment_row[0:1, HB + 128 * mb:HB + 128 * (mb + 1)],
                                    idn[0:1, 0:1])
                mc = cp.tile([128, 1], F32, tag=f"mcol{mb}")
                nc.scalar.copy(mc[:], pst[:])
                ment_col.append(mc)

            # ---- Ya (owned window, + b1p) and Yb (full window) ----
            ya = []
            yb = []
            for h, (ho, hn) in enumerate(hsl):
                psya = pp_pre.tile([hn, MC], F32, tag="pre_ps")
                for t in range(NDT):
                    nc.tensor.matmul(psya[:], w1a_s[t][:, ho:ho + hn], xts[t][:, HB:W],
                                     start=(t == 0), stop=(t == NDT - 1))
                y = cp.tile([hn, MC], F16, tag=f"ya{h}")
                nc.scalar.activation(y[:], psya[:], AF.Identity, bias=b1p_c[h][:])
                ya.append(y)
                psyb = pp_pre.tile([hn, W], F32, tag="pre_ps")
                for t in range(NDT):
                    nc.tensor.matmul(psyb[:], w1b_s[t][:, ho:ho + hn], xts[t][:],
                                     start=(t == 0), stop=(t == NDT - 1))
                y = cp.tile([hn, W], F16, tag=f"yb{h}")
                nc.scalar.copy(y[:], psyb[:])
                yb.append(y)

            # ---- banded pairwise loop: G deltas per round ----
            GW = G * MC
            pair_flat = cp.tile([1, KM], F16, tag="pairflat")
            a2x_bufs = []
            for i in range(2):
                ab = cp.tile([33, GW], F16, tag=f"a2x{i}")
                nc.vector.memset(ab[:], 0.0)
                a2x_bufs.append(ab)
            for r in range(NR):
                d0 = 1 + G * r  # deltas d0 .. d0+G-1

                # products P[d, j, m] = X^T[d, m] * X^T[d, m - (d0+j)]
                pts = []
                for t in range(NDT):
                    pt = wp.tile([128, GW], F16, tag=f"p{t}")
                    nc.vector.tensor_tensor(
                        _ap3(pt[:], 0, 128, 0, [(MC, G), (1, MC)]),
                        _ap3(xts[t][:], 0, 128, HB, [(0, G), (1, MC)]),
                        _ap3(xts[t][:], 0, 128, HB - d0, [(-1, G), (1, MC)]),
                        mybir.AluOpType.mult)
                    pts.append(pt)

                # C[h, j, m] = Ya'[h, m] + Yb[h, m - (d0+j)]
                c1 = wp.tile([H1, GW], F16, tag="c1")
                nc.vector.tensor_tensor(
                    _ap3(c1[:], 0, H1, 0, [(MC, G), (1, MC)]),
                    _ap3(ya[0][:], 0, H1, 0, [(0, G), (1, MC)]),
                    _ap3(yb[0][:], 0, H1, HB - d0, [(-1, G), (1, MC)]),
                    mybir.AluOpType.add)
                c2 = wp.tile([H2, GW], F16, tag="c2")
                nc.vector.tensor_tensor(
                    _ap3(c2[:], 0, H2, 0, [(MC, G), (1, MC)]),
                    _ap3(ya[1][:], 0, H2, 0, [(0, G), (1, MC)]),
                    _ap3(yb[1][:], 0, H2, HB - d0, [(-1, G), (1, MC)]),
                    mybir.AluOpType.add)

                # A = P @ W1c + C, per h tile, PSUM-accumulated
                ps1 = pp_a1.tile([H1, GW], F32, tag="a1")
                for t in range(NDT):
                    nc.tensor.matmul(ps1[:], w1c_s[t][:, 0:H1], pts[t][:],
                                     start=(t == 0), stop=False)
                nc.tensor.matmul(ps1[:], idn[0:H1, 0:H1], c1[:], start=False, stop=True)
                ps2 = pp_a2.tile([H2, GW], F32, tag="a2")
                for t in range(NDT):
                    nc.tensor.matmul(ps2[:], w1c_s[t][:, H1:H], pts[t][:],
                                     start=(t == 0), stop=False)
                nc.tensor.matmul(ps2[:], idn[0:H2, 0:H2], c2[:], start=False, stop=True)

                # relu evacuation (b1p already inside C via Ya)
                a1 = wp.tile([H1, GW], F16, tag="a1sb")
                nc.scalar.activation(a1[:], ps1[:], AF.Relu)
                a2x = a2x_bufs[r % 2]
                nc.scalar.activation(a2x[0:H2, :], ps2[:], AF.Relu)
                # ment_j carrier row (at partition 32; w2p2 rows 22..31 are 0)
                nc.scalar.copy(
                    _ap3(a2x[:], 32, 1, 0, [(MC, G), (1, MC)]),
                    _ap3(ment_row[:], 0, 1, HB - d0, [(-1, G), (1, MC)]))

                # pair = w2p . A  (+ ment_j via carrier)
                psp = pp_pair.tile([1, GW], F32, tag="pair")
                nc.tensor.matmul(psp[:], w2p1[:], a1[:], start=True, stop=False)
                nc.tensor.matmul(psp[:], w2p2[:], a2x[:], start=False, stop=True)
                for j in range(G):
                    k = K - (d0 + j)
                    nc.scalar.copy(pair_flat[0:1, MC * k:MC * (k + 1)],
                                   psp[0:1, MC * j:MC * (j + 1)])

            # ---- respread (k-major) to rows, transpose, mask, store ----
            pairK = cp.tile([K, MC], F16, tag="pairK")
            nc.sync.dma_start(
                out=pairK[:],
                in_=_ap3(pair_flat[:], 0, 1, 0, [(MC, K), (1, MC)]))
            for mb in range(2):
                pst = pp_pre.tile([128, K], F16, tag="pre_ps")
                nc.tensor.transpose(pst[:], pairK[:, 128 * mb:128 * (mb + 1)],
                                    idn[0:K, 0:K])
                sc = wp.tile([128, K + 1], F32, tag=f"sc{mb}")
                nc.vector.memset(sc[:], 0.0)
                nc.scalar.activation(sc[:, 0:K], pst[:], AF.Identity,
                                     bias=ment_col[mb][:])
                nc.vector.tensor_mul(sc[:], sc[:], mm_sb[mb][:])
                nc.vector.tensor_add(sc[:], sc[:], ma_sb[mb][:])
                nc.sync.dma_start(out=out_d[128 * mb:128 * (mb + 1), :], in_=sc[:])

    nc.compile()
    return nc


def _prep_inputs(inputs):
    X = np.ascontiguousarray(inputs["mention_reprs"], dtype=np.float32)
    assert X.shape == (M, D)
    w1p = np.asarray(inputs["w1p"], dtype=np.float32)
    W1a, W1b, W1c = w1p[:D], w1p[D:2 * D], w1p[2 * D:]
    f16 = lambda a: np.ascontiguousarray(a, dtype=np.float16)

    def padD(w):  # [D, H] -> [DP, H] fp16
        out = np.zeros((DP, H), dtype=np.float16)
        out[:D] = w.astype(np.float16)
        return out

    xtp = np.zeros((DP, M + HB), dtype=np.float16)
    xtp[:D, HB:] = X.T.astype(np.float16)

    w2p = np.asarray(inputs["w2p"], dtype=np.float32)
    shared = {
        "w1c": padD(W1c),
        "w1a": padD(W1a),
        "w1b": padD(W1b),
        "w1m": padD(np.asarray(inputs["w1m"], dtype=np.float32)),
        "w2m": f16(np.asarray(inputs["w2m"], dtype=np.float32).reshape(H, 1)),
        "w2p1": f16(w2p[:H1].reshape(H1, 1)),
        "w2p2": f16(np.concatenate([w2p[H1:], np.zeros(10, np.float32),
                                    [1.0]]).reshape(33, 1)),
        "idn": np.eye(128, dtype=np.float16),
        "b1mc": np.ascontiguousarray(
            np.asarray(inputs["b1m"], dtype=np.float32).reshape(H, 1)),
        "b1pc": np.ascontiguousarray(
            np.asarray(inputs["b1p"], dtype=np.float32).reshape(H, 1)),
    }

    b2m = float(np.asarray(inputs["b2m"]).reshape(-1)[0])
    b2p = float(np.asarray(inputs["b2p"]).reshape(-1)[0])
    in_maps = []
    for c in range(NCORES):
        r0 = MC * c
        xt_c = np.ascontiguousarray(xtp[:, r0:r0 + W])
        mmul = np.ones((MC, K + 1), dtype=np.float32)
        madd = np.full((MC, K + 1), np.float32(b2p + 2.0 * b2m), dtype=np.float32)
        mmul[:, K] = 0.0
        madd[:, K] = 0.0
        if c == 0:
            for i in range(min(K, MC)):
                mmul[i, :K - i] = 0.0
                madd[i, :K - i] = np.float32(-1e9)
        in_maps.append({"xt": xt_c, "mmul": mmul, "madd": madd, **shared})
    return in_maps


def _get_nc(inputs):
    if "nc" not in _cache:
        _cache["nc"] = _build()
    return _cache["nc"]


def _run(inputs, trace=False):
    assert int(np.asarray(inputs["K"])) == K
    nc = _get_nc(inputs)
    in_maps = _prep_inputs(inputs)
    res = run_bass_kernel_spmd(nc, in_maps, list(range(NCORES)), trace=trace)
    out = np.concatenate([res.results[c]["out"] for c in range(NCORES)], axis=0)
    return out.astype(np.float32), res


def kernel(**inputs) -> np.ndarray:
    out, _ = _run(inputs, trace=False)
    return out
